# revision 46
# baseline (speedup 1.0000x reference)
"""Multi-head attention (B=2, T=2048, E=1024, H=16) on 8 TRN2 NeuronCores.

Sharding: core c handles batch c//4 and head group c%4 (4 heads of 64 dims
-> 256 columns of w_Q/w_K/w_V and of the output). Pure SPMD, no collectives:
every core runs the same NEFF on its own input shard.

Per-core kernel (all matmul operands bf16, PSUM/softmax math fp32):
  xT [E, T] (host pre-transposed), wq/wk/wv [E, 256]
  1. QT/KT per head-pair p: [128, T] = (w pair-slice)^T @ xT   (PE)
  2. V per s-tile: [128, 4*65] with a ones column per head     (PE + DVE copy)
  3. scores transposed per head: ST[s, t] = K Q^T, two heads packed into
     PE row groups (K=64 each) writing one [128, 1024] PSUM tile per s-tile
  4. exp: per s-tile group, either ACT exp straight from PSUM (scale=1/8
     folded into the activation affine, bf16 out -> PT) or DVE Schraudolph
     2^x bit trick writing the top-16 f32 bits (= bf16 pattern) straight
     into PT via an int16 bitcast; the two engines alternate across groups
     (DVE_SETS) so the PSUM score buffers (3 bufs) drain in parallel
  5. attn: out[t,65] = PT_slice^T @ V_aug accumulated over 16 s-chunks;
     col 64 = softmax denominator (from the ones column)
  6. normalize: one strided DVE reciprocal [128,4] + one broadcast
     tensor_tensor mul -> fp32 staging tile -> one fused out-DMA per unit

Scheduling (program order = Tile scheduler priority):
  - attn work of unit u is emitted as 11 filler chunks interleaved between
    the score s-tiles of unit u+2, so PE always has queued work while
    scores wait for exp to free PSUM bufs
  - projection hooks are placed >=2 s-tiles before their first consumer
  - attn h1 borrows the idle proj PSUM bank for units >= 3; early proj
    hooks alternate proj/attn banks for double buffering
  - the ACT exp table is warmed at t=0; input DMAs arrive in first-use
    order (wq, xT tc0, wk, ...); in an R-reps timing NEFF consecutive
    bodies overlap ~10us of each other's startup/tail
"""

import numpy as np
import ml_dtypes

B, T, E, H = 2, 2048, 1024, 16
D = 64          # head dim
HG = 4          # heads per core
GC = HG * D     # 256 output columns per core
NCORES = 8

_cached_nc = None
_PHASE_LOG = []  # (tag, next-instruction-index) marks when phase_marks=True


def _build_program(seq: int = T, reps: int = 1, skip_attn=False, skip_exp=False,
                   phase_marks=False):
    """reps>1 emits the body multiple times in one NEFF (timing only).
    skip_attn/skip_exp build ablation variants for HW phase attribution."""
    import concourse.bacc as bacc
    import concourse.tile as tile
    from concourse import mybir

    bf16 = mybir.dt.bfloat16
    f32 = mybir.dt.float32
    i16 = mybir.dt.int16
    Exp = mybir.ActivationFunctionType.Exp
    Mult = mybir.AluOpType.mult
    Add = mybir.AluOpType.add
    # Schraudolph fast-exp constants (2^x bit trick), with the 1/sqrt(HD)
    # score scale folded into the multiplier like the ACT path's scale=.
    # Scaled by 2^-16 so the rounded result is the TOP 16 bits of the f32
    # pattern = the bf16 pattern: DVE writes exp() straight into the bf16
    # pt tile via an int16 bitcast, no GPSIMD copy stage needed.
    SCH_C1 = (1 << 7) * 1.4426950408889634 * 0.125
    SCH_C2 = (1 << 7) * (127.0 - 0.04367744)
    # Per-unit set of exp groups (one group = one s-tile = 2 PSUM banks)
    # handled by DVE (sch); the rest go to ACT.  Alternating engines across
    # consecutive groups keeps both drains running in parallel, and 3 score
    # bufs give the pipeline a reuse distance that covers the drain latency.
    DVE_SETS = (({3, 6, 9, 12, 15},) * 2 + ({3, 5, 7, 9, 11, 13, 15},) * 4
                + ({3, 6, 9, 12, 15},) * 2)

    NT = seq // 128     # s-tiles / t-tiles
    NTC = seq // 512    # 512-wide t-chunks
    KO = E // 128       # contraction chunks for projections

    nc = bacc.Bacc(
        "TRN2", target_bir_lowering=False, debug=False, num_devices=NCORES
    )

    _PHASE_LOG.clear()

    def mark(tag):
        if phase_marks:
            _PHASE_LOG.append(
                (tag, int(nc.get_next_instruction_name().split("-")[1]))
            )

    xT_d = nc.dram_tensor("xT", [E, seq], bf16, kind="ExternalInput")
    wq_d = nc.dram_tensor("wq", [E, GC], bf16, kind="ExternalInput")
    wk_d = nc.dram_tensor("wk", [E, GC], bf16, kind="ExternalInput")
    wv_d = nc.dram_tensor("wv", [E, GC], bf16, kind="ExternalInput")
    out_d = nc.dram_tensor("out", [seq, GC], f32, kind="ExternalOutput")

    with tile.TileContext(nc) as tc:
        with (
            tc.tile_pool(name="singles", bufs=1) as singles,
            tc.tile_pool(name="pt", bufs=3) as ptp,
            tc.tile_pool(name="stage", bufs=3) as stagep,
            tc.tile_pool(name="recip", bufs=8) as recipp,
            # PSUM budget (8 banks): scores 3x[128,1024] (6) + attn 1 + proj 1
            tc.tile_pool(name="proj_ps", bufs=1, space="PSUM") as proj_ps,
            tc.tile_pool(name="score_ps", bufs=3, space="PSUM") as score_ps,
            tc.tile_pool(name="attn_ps", bufs=1, space="PSUM") as attn_ps,
        ):
          for _rep in range(reps):
            # Warm the ACT exp table at t=0 so the first real exp doesn't
            # pay the ~2.7us LoadActFuncSet on the critical path.  (No PE
            # warm-up matmuls: in the steady state bodies overlap and PE
            # never idles at a body boundary, so they would be pure waste.)
            warm = singles.tile([128, 2], f32)
            nc.vector.memset(warm[:], 0.0)
            nc.scalar.activation(out=warm[:, 1:2], in_=warm[:, 0:1], func=Exp)

            # ---- load inputs, first-matmul operands first: the wq-proj of
            # t-chunk 0 consumes wq[k] + xT[k, 0:512] in k order.  Bulk
            # rearranged transfers keep >=2KB per partition line.
            wq = singles.tile([128, KO, GC], bf16)
            wk = singles.tile([128, KO, GC], bf16)
            wv = singles.tile([128, KO, GC], bf16)
            xT = singles.tile([128, KO, seq], bf16)
            wqv = wq_d[:].rearrange("(ko p) c -> p ko c", p=128)
            wkv = wk_d[:].rearrange("(ko p) c -> p ko c", p=128)
            xTv = xT_d[:].rearrange("(ko p) c -> p ko c", p=128)
            nc.sync.dma_start(wq[:, 0:4], wqv[:, 0:4])
            nc.sync.dma_start(xT[:, 0:4, 0:512], xTv[:, 0:4, 0:512])
            nc.sync.dma_start(wk[:, 0:4], wkv[:, 0:4])
            nc.sync.dma_start(wq[:, 4:8], wqv[:, 4:8])
            nc.sync.dma_start(xT[:, 4:8, 0:512], xTv[:, 4:8, 0:512])
            nc.sync.dma_start(wk[:, 4:8], wkv[:, 4:8])
            wvv = wv_d[:].rearrange("(ko p) c -> p ko c", p=128)
            for tcq in range(1, NTC):
                c0, c1 = tcq * 512, (tcq + 1) * 512
                nc.sync.dma_start(xT[:, 0:4, c0:c1], xTv[:, 0:4, c0:c1])
                nc.sync.dma_start(xT[:, 4:8, c0:c1], xTv[:, 4:8, c0:c1])
                if tcq == 2:
                    nc.sync.dma_start(wv[:, 0:4], wvv[:, 0:4])
            nc.sync.dma_start(wv[:, 4:8], wvv[:, 4:8])

            # QT/KT: [128, pair, seq]; partitions 0-63 head 2p, 64-127 head 2p+1
            QT = singles.tile([128, 2, seq], bf16)
            KT = singles.tile([128, 2, seq], bf16)
            # V with ones col per head: [128, s-tile, 4*65]
            V = singles.tile([128, NT, HG * (D + 1)], bf16)
            nc.gpsimd.memset(V[:], 1.0)

            def proj_qk(p, w_sb, dst, tcs, pool=None, split_copy=False,
                        alt=False):
                """Project t-chunks `tcs` of QT or KT for head-pair p.
                alt=True alternates the psum bank with the (idle) attn bank
                for double buffering during units 0-1."""
                for i_, tcq in enumerate(tcs):
                    mark(f"projqk p{p} tc{tcq}")
                    use_attn = (pool is not None) or (alt and i_ % 2 == 0)
                    ps = (attn_ps if use_attn else proj_ps).tile(
                        [128, 512], f32,
                        tag="attn" if use_attn else "proj", name="ps")
                    for k in range(KO):
                        nc.tensor.matmul(
                            ps[:],
                            lhsT=w_sb[:, k, p * 128:(p + 1) * 128],
                            rhs=xT[:, k, tcq * 512:(tcq + 1) * 512],
                            start=(k == 0),
                            stop=(k == KO - 1),
                        )
                    base = tcq * 512
                    if split_copy:
                        # ACT copies the first s-tile so scores start sooner
                        nc.scalar.copy(
                            out=dst[:, p, base:base + 128], in_=ps[:, 0:128])
                        nc.vector.tensor_copy(
                            out=dst[:, p, base + 128:base + 512],
                            in_=ps[:, 128:512])
                    else:
                        nc.vector.tensor_copy(
                            out=dst[:, p, base:base + 512], in_=ps[:]
                        )

            def proj_v(tiles, alt=False):
                for i_, st in enumerate(tiles):
                    mark(f"projv {st}")
                    use_attn = alt and i_ % 2 == 0
                    ps = (attn_ps if use_attn else proj_ps).tile(
                        [128, 512], f32,
                        tag="attn" if use_attn else "proj", name="ps")
                    for k in range(KO):
                        nc.tensor.matmul(
                            ps[:, :GC],
                            lhsT=xT[:, k, st * 128:(st + 1) * 128],
                            rhs=wv[:, k, :],
                            start=(k == 0),
                            stop=(k == KO - 1),
                        )
                    nc.vector.tensor_copy(
                        out=V[:, st].rearrange("p (h c) -> p h c", h=HG)[:, :, :D],
                        in_=ps[:, :GC].rearrange("p (h c) -> p h c", h=HG),
                    )

            def scores_unit(p, tcq, hooks=None, dve_set=frozenset(), fill=None,
                            uid=0):
                """ST = K Q^T (both heads row-packed) + exp -> PT tile.

                PT layout is flat [128, NT*1024]: 512-wide bank-write j=2*st+h
                lands at elem offset j*512 (= st*1024 + h*512). Exps are
                grouped 3 banks at a time (FD=1536) to amortize the ACT
                per-op PSUM overhead. hooks[st] emits filler work just
                before score s-tile st; `fill` is a deque of filler chunk
                emitters (attn work of the unit two back) drained one per odd
                s-tile so PE has queued work while the score matmuls wait on
                exp to free PSUM score buffers."""
                pt = ptp.tile([128, NT * 1024], bf16, tag="pt")
                for st in range(NT):
                    for f in (hooks or {}).get(st, []):
                        f()
                    if fill and st >= 1:
                        fill.popleft()()
                    mark(f"sc u{uid} g{st}")
                    sc = score_ps.tile([128, 1024], f32, tag="score")
                    for h in range(2):
                        nc.tensor.matmul(
                            sc[:, h * 512:(h + 1) * 512],
                            lhsT=KT[h * 64:(h + 1) * 64, p,
                                    st * 128:(st + 1) * 128],
                            rhs=QT[h * 64:(h + 1) * 64, p,
                                   tcq * 512:(tcq + 1) * 512],
                            start=True,
                            stop=True,
                        )
                    if skip_exp:
                        continue
                    dst = pt[:, st * 1024:(st + 1) * 1024]
                    # DVE handles `dve_set` groups via the Schraudolph 2^x
                    # bit trick, writing the top-16 bits of the f32 pattern
                    # (= the bf16 pattern) straight into pt; ACT does a real
                    # exp on the rest.
                    if st in dve_set:
                        nc.vector.tensor_scalar(
                            dst.bitcast(i16), sc[:],
                            SCH_C1, SCH_C2, Mult, Add,
                        )
                    else:
                        nc.scalar.activation(
                            out=dst, in_=sc[:], func=Exp, scale=0.125,
                        )
                while fill:
                    fill.popleft()()
                return pt

            def attn_chunks(p, tcq, pt, uid=0, last=False):
                """attn = PT^T @ V_aug accumulated over s, then normalize.
                Returns a list of emitter chunks (4 matmul chains + a
                normalize per head, then the out-DMAs) so the caller can
                interleave them between score groups as PE filler.  Heads
                run sequentially so one PSUM accumulator bank suffices."""
                state = {}

                def chain(h, tt):
                    def f():
                        mark(f"at u{uid} h{h} tt{tt}")
                        if tt == 0:
                            if h == 0:
                                state["stg"] = stagep.tile(
                                    [128, 4, 128], f32, tag="stage",
                                    name="stg")
                            # after the proj phase (units >= 3) the two
                            # heads alternate between the attn bank and the
                            # idle proj bank, so a chain never waits for the
                            # previous normalize to release its accumulator
                            if uid >= 3 and (2 * uid + h) % 2 == 0:
                                state["ap"] = proj_ps.tile(
                                    [128, 4 * (D + 1)], f32, tag="proj",
                                    name="ap")
                            else:
                                state["ap"] = attn_ps.tile(
                                    [128, 4 * (D + 1)], f32, tag="attn",
                                    name="ap")
                        ap = state["ap"]
                        hh = p * 2 + h
                        for st in range(NT):
                            nc.tensor.matmul(
                                ap[:, tt * (D + 1):(tt + 1) * (D + 1)],
                                lhsT=pt[:, st * 1024 + h * 512 + tt * 128:
                                        st * 1024 + h * 512 + (tt + 1) * 128],
                                rhs=V[:, st, hh * (D + 1):(hh + 1) * (D + 1)],
                                start=(st == 0),
                                stop=(st == NT - 1),
                            )
                    return f

                def norm(h):
                    def f():
                        mark(f"nm u{uid} h{h}")
                        ap, stg = state["ap"], state["stg"]
                        # one strided reciprocal for all 4 denominators, one
                        # broadcast multiply for all 4 t-tiles
                        r4 = recipp.tile([128, 4], f32, tag="recip")
                        apv = ap[:].rearrange("q (tt c) -> q tt c", tt=4)
                        nc.vector.reciprocal(out=r4[:], in_=apv[:, :, D])
                        nc.vector.tensor_tensor(
                            out=stg[:, :, h * D:(h + 1) * D],
                            in0=apv[:, :, 0:D],
                            in1=r4[:].rearrange("q (tt o) -> q tt o", o=1)
                                     .broadcast_to([128, 4, D]),
                            op=Mult,
                        )
                    return f

                def out():
                    def f():
                        stg = state["stg"]
                        nc.sync.dma_start(
                            out_d[tcq * 512:(tcq + 1) * 512,
                                  p * 128:(p + 1) * 128]
                            .rearrange("(tt q) c -> q tt c", q=128),
                            stg[:],
                        )
                    return f

                skip = lambda: None
                if uid < 3:
                    # shared accumulator bank: give norm(h0) a score s-tile
                    # of breathing room before the h1 chain reclaims the bank
                    return [chain(0, 0), chain(0, 1), chain(0, 2),
                            chain(0, 3), norm(0), skip, chain(1, 0),
                            chain(1, 1), chain(1, 2), chain(1, 3), norm(1),
                            out()]
                return [chain(0, 0), chain(0, 1), chain(0, 2), chain(0, 3),
                        norm(0), chain(1, 0), chain(1, 1), chain(1, 2),
                        chain(1, 3), norm(1), out()]

            # Program order is semantic order under Tile (WAR/RAW follow it),
            # and it is also the scheduler's priority order. Software-pipeline
            # the softmax: emit scores(u+1) before attn(u) so ACT never
            # starves at a unit boundary; slot filler work (V projection,
            # pair-1 QK, deferred QT-0 chunks) right after the scores that
            # precede its first use.
            # Minimal critical path to the first exp: QT0[tc0], KT0[tc0],
            # then unit-0 scores. All remaining projection work (KT0 tails,
            # QT0 tails, V, pair-1 QK) is spread through the score s-loops
            # as hook filler so PE keeps ACT fed instead of lumping
            # projections between units. attn runs two units behind scores
            # (pt pool bufs >= 3). Everything is emitted before its first
            # program-order use (Tile semantics follow program order).
            proj_qk(0, wq, QT, [0], pool=attn_ps)
            proj_qk(0, wk, KT, [0], split_copy=True)
            units = [(p, tcq) for p in range(2) for tcq in range(NTC)]
            qk0 = lambda w, d, tcs, alt=False: (
                lambda: proj_qk(0, w, d, tcs, alt=alt))
            qk1 = lambda w, d, tcs: (lambda: proj_qk(1, w, d, tcs))
            pv = lambda ts, alt=False: (lambda: proj_v(ts, alt=alt))
            if NTC == 4:
                hooks = {
                    0: {2: [qk0(wk, KT, [1])], 6: [qk0(wk, KT, [2], alt=True)],
                        10: [qk0(wk, KT, [3])], 13: [qk0(wq, QT, [1], alt=True)]},
                    1: {0: [pv(range(0, 4), alt=True)],
                        3: [pv(range(4, 8), alt=True)],
                        7: [qk0(wq, QT, [2], alt=True)],
                        11: [pv(range(8, 12), alt=True)],
                        14: [pv(range(12, 16), alt=True)]},
                    2: {0: [qk0(wq, QT, [3])], 4: [qk1(wk, KT, [0])],
                        8: [qk1(wk, KT, [1])], 12: [qk1(wk, KT, [2])]},
                    3: {0: [qk1(wk, KT, [3])], 4: [qk1(wq, QT, [0])],
                        8: [qk1(wq, QT, [1])], 12: [qk1(wq, QT, [2])]},
                    4: {0: [qk1(wq, QT, [3])]},
                }
                fillers = {}
            else:
                hooks = {0: {4 * c: [qk0(wk, KT, [c])] for c in range(1, NTC)}}
                fillers = {0: [qk0(wq, QT, range(1, NTC)),
                               pv(range(NT))]}
                fillers.setdefault(min(1, NTC - 1), []).extend((
                    qk1(wk, KT, range(NTC)),))
                fillers.setdefault(min(2, NTC - 1), []).append(
                    qk1(wq, QT, range(NTC)))
            from collections import deque

            pending = []  # [(i, p, tcq, pt)] up to two units behind
            for i, (p, tcq) in enumerate(units):
                fill = None
                if len(pending) == 2 and not skip_attn:
                    fill = deque(attn_chunks(*pending.pop(0)))
                elif len(pending) == 2:
                    pending.pop(0)
                pt = scores_unit(p, tcq, hooks.get(i),
                                 dve_set=DVE_SETS[i % len(DVE_SETS)],
                                 fill=fill, uid=i)
                for f in fillers.get(i, []):
                    f()
                pending.append((p, tcq, pt, i))
            for n, args in enumerate(pending):
                if not skip_attn:
                    for f in attn_chunks(*args, last=(n == len(pending) - 1)):
                        f()

    nc.compile()
    return nc


def _shard_inputs(x, w_Q, w_K, w_V):
    bf = ml_dtypes.bfloat16
    in_maps = []
    for c in range(NCORES):
        b, g = divmod(c, NCORES // B)
        cols = slice(g * GC, (g + 1) * GC)
        in_maps.append({
            "xT": np.ascontiguousarray(np.asarray(x)[b].T).astype(bf),
            "wq": np.ascontiguousarray(np.asarray(w_Q)[:, cols]).astype(bf),
            "wk": np.ascontiguousarray(np.asarray(w_K)[:, cols]).astype(bf),
            "wv": np.ascontiguousarray(np.asarray(w_V)[:, cols]).astype(bf),
        })
    return in_maps


def kernel(x, w_Q, w_K, w_V, _trace=False, _tmpdir=None):
    from concourse.bass_utils import run_bass_kernel_spmd

    global _cached_nc
    if _cached_nc is None:
        _cached_nc = _build_program(T)
    in_maps = _shard_inputs(x, w_Q, w_K, w_V)
    res = run_bass_kernel_spmd(
        _cached_nc, in_maps, list(range(NCORES)),
        trace=_trace, tmpdir=_tmpdir,
    )
    out = np.empty((B, T, E), np.float32)
    for c in range(NCORES):
        b, g = divmod(c, NCORES // B)
        out[b, :, g * GC:(g + 1) * GC] = res.results[c]["out"]
    if _trace:
        return out, res
    return out

